# revision 34
# baseline (speedup 1.0000x reference)
"""Trainium2 Bass kernel for BaseAttention (B=4, S=2048, H=16 heads x 64).

Sharding: 8 cores = 4 batches x 2 head-groups (8 heads / 512 dims each).
Each core computes q/k/v projections for its head group on its batch,
flash-style causal attention (scores never leave the chip), and a partial
o-projection over its 512 head dims. The host sums the two partial bf16
outputs per batch.

All matmul operands are bf16 (fp32 PSUM accumulation; rel err ~5e-3):
bf16 enables the PE fast-weight-load path and halves DMA traffic.
Key structure:
- attn^T stays resident in SBUF; odd heads reach partitions 64:128 via a
  small DRAM bounce (DVE ops cannot cross partitions, and the custom DVE
  reciprocal corrupts SBUF when issued at base partition 64 - keep the
  reciprocal on partitions 0:64, after the ones-row broadcast matmul).
- Dedicated PSUM rings per phase (scores 2x[P,1024], proj/oproj/bcast
  2x[P,512], PV accumulators 2x[65,512]) decouple the pipelines.
- Diagonal score blocks trim the fully-masked query range out of the
  score/exp/PV instructions (~50% of diagonal work).
- PV matmuls are deferred three pairs behind the scores to hide the exp
  latency; accumulators are evacuated to SBUF immediately at head-pair end
  so the next pair's PSUM writes never wait on the normalization chain.
- Startup weight DMAs ride the Activation DGE queue in parallel with the
  x^T loads on the SP queue.
"""

import numpy as np

B = 4
S = 2048
HIDDEN = 1024
NH = 16
DH = 64
HG = 2                  # head groups (cores per batch)
DG = HIDDEN // HG       # 512 dims per group (8 heads)
NCORES = B * HG
SCALE = DH ** -0.5

P = 128
KC = HIDDEN // P        # 8 contraction chunks for projections
NQ = S // 512           # 4 query chunks of 512
SM = S // P             # 16 seq chunks of 128
MCH = DG // P           # 4 chunks of 128 over the group's 512 dims
NHG = NH // HG          # 8 heads per core
NJ = NHG // 2           # 4 head pairs per core

_CACHE = {}


def _emit(nc, tc, tens):
    import concourse.mybir as mybir
    import concourse.bass as bass
    from collections import deque
    from contextlib import ExitStack

    f32 = mybir.dt.float32
    bf16 = mybir.dt.bfloat16
    Exp = mybir.ActivationFunctionType.Exp
    mult = mybir.AluOpType.mult
    ds = bass.ds

    xT = tens["xT"].ap().rearrange("(kc p) s -> p kc s", p=P)
    wqT = tens["wqT"].ap().rearrange("(kc p) d -> p kc d", p=P)
    wkT = tens["wkT"].ap().rearrange("(kc p) d -> p kc d", p=P)
    wvT = tens["wvT"].ap().rearrange("(kc p) d -> p kc d", p=P)
    woT = tens["woT"].ap().rearrange("(ic p) j -> p ic j", p=P)
    masks = tens["masks"].ap().rearrange("t p q -> p t q")
    out = tens["out"].ap().rearrange("(sm p) j -> p sm j", p=P)

    with ExitStack() as ctx:
        persist = ctx.enter_context(tc.tile_pool(name="persist", bufs=1))
        dram = ctx.enter_context(tc.tile_pool(name="dram", bufs=1, space="DRAM"))
        ps_sc = ctx.enter_context(tc.tile_pool(name="ps_sc", bufs=2, space="PSUM"))
        ps_pj = ctx.enter_context(tc.tile_pool(name="ps_pj", bufs=2, space="PSUM"))
        ps_at = ctx.enter_context(tc.tile_pool(name="ps_at", bufs=2, space="PSUM"))
        pstage = ctx.enter_context(tc.tile_pool(name="pstage", bufs=2))
        ptp = ctx.enter_context(tc.tile_pool(name="pt", bufs=8))
        astp = ctx.enter_context(tc.tile_pool(name="ast", bufs=2))
        recp = ctx.enter_context(tc.tile_pool(name="rec", bufs=2))
        ostp = ctx.enter_context(tc.tile_pool(name="ost", bufs=3))
        qpool = ctx.enter_context(tc.tile_pool(name="qp", bufs=2))

        kT_sb = persist.tile([P, MCH, S], bf16)          # k^T (d on partitions)
        v_sb = persist.tile([P, SM, NHG, DH + 1], bf16)  # v + ones column
        ones_sb = persist.tile([P, DH], bf16)
        wq_sb = persist.tile([P, KC, DG], bf16)
        wk_sb = persist.tile([P, KC, DG], bf16)
        wv_sb = persist.tile([P, KC, DG], bf16)
        wo_sb = persist.tile([P, MCH, HIDDEN], bf16)
        mask_sb = persist.tile([P, 2, 1024], mybir.dt.bfloat16)
        attnT_sb = persist.tile([P, MCH, S], bf16)       # attn^T resident
        ast_d = dram.tile([NQ, NJ, DH, 512], bf16)       # odd-head bounce



        ones_f32 = persist.tile([P, 1], f32)
        nc.vector.memset(ones_f32[:], 1.0)  # bf16 memset fails ISA checks
        nc.vector.tensor_copy(out=ones_sb[:], in_=ones_f32[:, 0:1].to_broadcast([P, DH]))
        nc.vector.tensor_copy(
            out=v_sb[:, :, :, DH:DH + 1],
            in_=ones_f32[:, 0:1].to_broadcast([P, SM, NHG, 1]),
        )

        xts = {}
        qsbs = {}

        def proj_closures(n):
            """q/k/v projection work for seq chunk n: 13 closures."""
            cls = []

            def load_xt(n=n):
                xt = pstage.tile([P, KC, 512], bf16, tag="xt")
                for hh in range(4):
                    hsl = ds(hh * (KC // 4), KC // 4)
                    nc.sync.dma_start(xt[:, hsl, :], xT[:, hsl, ds(n * 512, 512)])
                xts[n] = xt
                qsbs[n] = qpool.tile([P, MCH, 512], bf16, tag="qsb", name=f"qsb_{n}")
            cls.append(load_xt)

            for m in range(MCH):
                for w_sb, dst_name in ((wq_sb, "q"), (wk_sb, "k")):
                    def qk_group(n=n, w_sb=w_sb, dst_name=dst_name, m=m):
                        xt = xts[n]
                        ps = ps_pj.tile([P, 512], f32, tag="pj")
                        for kc in range(KC):
                            nc.tensor.matmul(
                                ps[:],
                                w_sb[:, kc, ds(m * P, P)],
                                xt[:, kc, :],
                                start=(kc == 0), stop=(kc == KC - 1),
                            )
                        if dst_name == "k":
                            nc.vector.tensor_copy(
                                out=kT_sb[:, m, ds(n * 512, 512)], in_=ps[:, :512]
                            )
                        else:
                            nc.vector.tensor_copy(
                                out=qsbs[n][:, m, :], in_=ps[:, :512]
                            )
                    cls.append(qk_group)

            for sm in range(4 * n, 4 * n + 4):
                def v_group(n=n, sm=sm):
                    xt = xts[n]
                    ps = ps_pj.tile([P, 512], f32, tag="pj")
                    for kc in range(KC):
                        nc.tensor.matmul(
                            ps[:],
                            xt[:, kc, ds((sm - 4 * n) * P, P)],
                            wv_sb[:, kc, :],
                            start=(kc == 0), stop=(kc == KC - 1),
                        )
                    nc.vector.tensor_copy(
                        out=v_sb[:, sm, :, 0:DH],
                        in_=ps[:].rearrange("p (h d) -> p h d", h=NHG),
                    )
                cls.append(v_group)
            return cls

        def oproj_closures(n):
            """o-projection for seq chunk n: 8 closures (two per sm)."""
            cls = []
            for sm in range(4 * n, 4 * n + 4):
                for j2 in range(2):
                    def o_group(sm=sm, j2=j2):
                        ps = ps_pj.tile([P, 512], f32, tag="pj")
                        for ic in range(MCH):
                            nc.tensor.matmul(
                                ps[:],
                                attnT_sb[:, ic, ds(sm * P, P)],
                                wo_sb[:, ic, ds(j2 * 512, 512)],
                                start=(ic == 0), stop=(ic == MCH - 1),
                            )
                        ost = ostp.tile([P, 512], bf16, tag="ost")
                        with nc.allow_low_precision(reason="bf16 partial out"):
                            nc.vector.tensor_copy(out=ost[:], in_=ps[:])
                        oeng = nc.scalar if j2 else nc.sync
                        oeng.dma_start(out[:, sm, ds(j2 * 512, 512)], ost[:])
                    cls.append(o_group)
            return cls

        # startup: xt(0)+wq first so the PE starts within a few us; the
        # remaining weight DMAs stream behind the first matmul groups.
        p0 = proj_closures(0)
        p0[0]()                                   # xt(0) + qsb alloc
        for c4 in range(4):
            csl = ds(c4 * (DG // 4), DG // 4)
            nc.scalar.dma_start(wq_sb[:, :, csl], wqT[:, :, csl])
        for c4 in range(4):
            csl = ds(c4 * (DG // 4), DG // 4)
            nc.scalar.dma_start(wk_sb[:, :, csl], wkT[:, :, csl])
        for c in p0[1:9]:                         # q/k groups (interleaved)
            c()
        for c4 in range(4):
            csl = ds(c4 * (DG // 4), DG // 4)
            nc.sync.dma_start(wv_sb[:, :, csl], wvT[:, :, csl])
        nc.sync.dma_start(mask_sb[:], masks)
        for c in p0[9:]:                          # v groups
            c()
        for c4 in range(4):
            eng = nc.sync if c4 % 2 else nc.scalar
            csl = ds(c4 * (HIDDEN // 4), HIDDEN // 4)
            eng.dma_start(wo_sb[:, :, csl], woT[:, :, csl])

        filler = deque()
        pending = []  # deferred normalization closures

        def flush_pending():
            for c in pending:
                c()
            pending.clear()

        def norm_closure(n, j, e, acc):
            h = 2 * j + e
            qsl = ds(n * 512, 512)
            # raw denominator row -> SBUF bf16 as the bcast-matmul rhs; the
            # reciprocal runs after the broadcast, on partitions 0:64 (the
            # custom DVE recip op corrupts SBUF when run at base partition 64)
            rec = recp.tile([DH + 1, 512], bf16, tag="rec")
            with nc.allow_low_precision(reason="denom row stage"):
                nc.vector.tensor_copy(out=rec[DH:DH + 1, :], in_=acc[DH:DH + 1, :])
            # evacuate attn rows too: frees the PSUM bank for the next head
            # pair without waiting for the deferred finish chain
            accsb = astp.tile([DH, 512], f32, tag="accsb")
            nc.vector.tensor_copy(out=accsb[:], in_=acc[0:DH, :])

            def finish():
                bc_full = ps_pj.tile([P, 512], f32, tag="pj", name=f"bc_{n}_{j}_{e}")
                bc = bc_full[0:DH, :]
                nc.tensor.matmul(bc, ones_sb[DH:DH + 1, :], rec[DH:DH + 1, :],
                                 start=True, stop=True)
                den = recp.tile([DH, 512], f32, tag="den")
                nc.vector.tensor_copy(out=den[:], in_=bc)
                rcp = recp.tile([DH, 512], f32, tag="rcp")
                nc.vector.reciprocal_approx_fast(rcp[:], den[:])
                with nc.allow_low_precision(reason="bf16 attn values"):
                    if e == 0:
                        nc.vector.tensor_tensor(
                            attnT_sb[0:DH, j, qsl], accsb[:], rcp[:], mult
                        )
                    else:
                        ast = astp.tile([DH, 512], bf16, tag="ast")
                        nc.vector.tensor_tensor(ast[:], accsb[:], rcp[:], mult)
                        # odd head: partitions 64:128 via DRAM bounce
                        nc.sync.dma_start(ast_d[n, j], ast[:])
                        nc.scalar.dma_start(attnT_sb[DH:P, j, qsl], ast_d[n, j])
            return finish

        for n in range(NQ):
            if n + 1 < NQ:
                pc = proj_closures(n + 1)
                pc[0]()                       # start xt(n+1) DMA immediately
                filler.extend(pc[1:])
            if n == 2:
                filler.extend(oproj_closures(0))
            elif n == 3:
                filler.extend(oproj_closures(1))
                filler.extend(oproj_closures(2))
            npairs = 2 * (n + 1)
            total_pairs = NJ * npairs
            pace_num = len(filler)
            pace_acc = 0
            qsl = ds(n * 512, 512)
            for j in range(NJ):
                acc = [
                    ps_at.tile([DH + 1, 512], f32, tag="acc",
                               name=f"acc_{n}_{j}_{e}")
                    for e in range(2)
                ]
                pvq = []  # deferred PV matmuls (consumed 2 pairs later)

                def emit_pv(pvq=pvq, acc=acc, npairs=npairs, j=j):
                    tp, e, u, pt, qo = pvq.pop(0)
                    h = 2 * j + e
                    m = 2 * tp + u
                    nc.tensor.matmul(
                        acc[e][:, ds(qo, 512 - qo)],
                        v_sb[:, m, h, :],
                        pt[:, ds(u * 512 + qo, 512 - qo)],
                        start=(tp == 0 and u == 0),
                        stop=(tp == npairs - 1 and u == 1),
                    )

                for t in range(npairs):
                    # pump interleaved proj/o-proj work in bursts of >=2
                    # groups: a dense >3.4us PE stretch lets the HAM clock
                    # gate open (scattered 1-group pumps never do)
                    pace_acc += pace_num
                    pops = 0
                    while pace_acc >= total_pairs and filler:
                        filler.popleft()()
                        pace_acc -= total_pairs
                        pops += 1
                    if n == 0 and pops < 2:
                        for _ in range(2 - pops):
                            if filler:
                                filler.popleft()()
                    new_pvq = []
                    # diagonal pairs: queries below the key chunk are fully
                    # masked; trim them out of the score/exp/mask/PV range
                    qoffs = [0, 0]
                    if t >= 2 * n:
                        qoffs = [(2 * (t - 2 * n) + u) * P for u in range(2)]
                    for e in range(2):          # head pair member
                        bp = e * DH             # base partition 0/64
                        ps = ps_sc.tile([P, 1024], f32, tag="sc")
                        for u in range(2):      # m-pair member
                            m = 2 * t + u
                            qo = qoffs[u]
                            nc.tensor.matmul(
                                ps[:, ds(u * 512 + qo, 512 - qo)],
                                kT_sb[bp:bp + DH, j, ds(m * P, P)],
                                qsbs[n][bp:bp + DH, j, ds(qo, 512 - qo)],
                                start=True, stop=True,
                            )
                            if pvq and pvq[0][0] <= t - 3:
                                emit_pv()
                        pt = ptp.tile([P, 1024], bf16, tag="pt")
                        if t >= 2 * n:
                            for u in range(2):
                                qo = qoffs[u]
                                usl = ds(u * 512 + qo, 512 - qo)
                                nc.scalar.activation(pt[:, usl], ps[:, usl],
                                                     Exp, scale=SCALE)
                                nc.vector.tensor_tensor(
                                    pt[:, usl], pt[:, usl],
                                    mask_sb[:, t - 2 * n, usl], mult,
                                )
                        else:
                            nc.scalar.activation(pt[:], ps[:], Exp, scale=SCALE)
                        if pvq and pvq[0][0] <= t - 3:
                            emit_pv()
                        new_pvq.extend((t, e, u, pt, qoffs[u]) for u in range(2))
                    while pvq and pvq[0][0] <= t - 3:
                        emit_pv()
                    pvq.extend(new_pvq)
                    if t >= 1 and pending:
                        flush_pending()
                while pvq:
                    emit_pv()
                for e in range(2):
                    pending.append(norm_closure(n, j, e, acc[e]))
            while filler:
                filler.popleft()()
        flush_pending()
        for c in oproj_closures(NQ - 1):
            c()


def _build():
    import concourse.mybir as mybir
    import concourse.tile as tile
    from concourse import bacc

    f32 = mybir.dt.float32
    bf16 = mybir.dt.bfloat16
    nc = bacc.Bacc("TRN2", target_bir_lowering=False, debug=False,
                   num_devices=NCORES)
    tens = {
        "xT": nc.dram_tensor("xT", [HIDDEN, S], bf16, kind="ExternalInput"),
        "wqT": nc.dram_tensor("wqT", [HIDDEN, DG], bf16, kind="ExternalInput"),
        "wkT": nc.dram_tensor("wkT", [HIDDEN, DG], bf16, kind="ExternalInput"),
        "wvT": nc.dram_tensor("wvT", [HIDDEN, DG], bf16, kind="ExternalInput"),
        "woT": nc.dram_tensor("woT", [DG, HIDDEN], bf16, kind="ExternalInput"),
        "masks": nc.dram_tensor("masks", [2, P, 1024], mybir.dt.bfloat16, kind="ExternalInput"),
        "out": nc.dram_tensor("out", [S, HIDDEN], bf16, kind="ExternalOutput"),
    }
    with tile.TileContext(nc) as tc:
        _emit(nc, tc, tens)
    nc.compile()
    return nc


def get_program():
    if "nc" not in _CACHE:
        _CACHE["nc"] = _build()
    return _CACHE["nc"]


def make_in_maps(hidden_states, attention_mask, wq, wk, wv, wo):
    """Build the per-core input maps (host-side sharding)."""
    hidden_states = np.asarray(hidden_states, dtype=np.float32)
    attention_mask = np.asarray(attention_mask, dtype=np.float32)
    wq = np.asarray(wq, dtype=np.float32)
    wk = np.asarray(wk, dtype=np.float32)
    wv = np.asarray(wv, dtype=np.float32)
    wo = np.asarray(wo, dtype=np.float32)

    # Pair-level mask tiles for the diagonal blocks of scores^T, derived from
    # the provided additive mask (0 = attend, big negative = blocked).
    # Tile [t][kk, 512u + qq] = allow(q = 512 + qq, k = 512 + (2t+u)*128 + kk).
    import ml_dtypes
    am = attention_mask[0, 0]
    mask_np = np.empty((2, P, 1024), dtype=np.float32)
    for t in range(2):
        for u in range(2):
            off = (2 * t + u) * P
            blk = (am[512:1024, 512 + off:512 + off + P] == 0.0)
            mask_np[t, :, u * 512:(u + 1) * 512] = blk.T.astype(np.float32)
    mask_np = mask_np.astype(ml_dtypes.bfloat16)

    in_maps = []
    for c in range(NCORES):
        b, g = divmod(c, HG)
        rows = slice(g * DG, (g + 1) * DG)
        bf = ml_dtypes.bfloat16
        in_maps.append({
            "xT": np.ascontiguousarray(hidden_states[b].T).astype(bf),
            "wqT": np.ascontiguousarray(wq[rows, :].T).astype(bf),
            "wkT": np.ascontiguousarray(wk[rows, :].T).astype(bf),
            "wvT": np.ascontiguousarray(wv[rows, :].T).astype(bf),
            "woT": np.ascontiguousarray(wo[:, rows].T).astype(bf),
            "masks": mask_np,
        })
    return in_maps


def combine_outputs(results):
    out = np.empty((B, S, HIDDEN), dtype=np.float32)
    for b in range(B):
        out[b] = (np.asarray(results[HG * b]["out"], dtype=np.float32)
                  + np.asarray(results[HG * b + 1]["out"], dtype=np.float32))
    return out


def kernel(hidden_states, attention_mask, wq, wk, wv, wo):
    from concourse.bass_utils import run_bass_kernel_spmd

    nc = get_program()
    in_maps = make_in_maps(hidden_states, attention_mask, wq, wk, wv, wo)
    res = run_bass_kernel_spmd(nc, in_maps, list(range(NCORES)))
    return combine_outputs(res.results)

